# revision 1
# baseline (speedup 1.0000x reference)
"""Trainium2 Bass kernel for AltAttention (B=2, S=2048, D=1024, 16 heads).

Distribution over 8 NeuronCores: data-parallel over batch (2) x
tensor-parallel over heads (4 heads/core). Each core computes, for its
(batch, head-group):
  qkvT projection (fp32r matmuls), scores^T = K^T-layout strips [k, q],
  softmax via exp on ScalarE + a ones-row appended to V (so the PV matmul
  also produces the softmax denominators), normalization, and a partial
  output projection. The host sums the 4 partial projections per batch and
  adds b_proj.

All matmul operands are fp16 (full PE rate, ~1.4e-4 input rounding;
PSUM accumulation stays fp32), giving ~4.8e-4 end-to-end relative error.
One 8-bank PSUM pool is shared by all phases (tags: accqk=4, sc=4 banks);
the program order software-pipelines each attention unit (scores one
k-tile ahead of the exp) and interleaves the QKV chains, V strips, unit
tails and the output projection into the units' spare PE slots via a
background work queue, so the ScalarE exp stream (the ~134us floor) runs
as continuously as possible.
"""
import numpy as np

import concourse.bacc as bacc
import concourse.mybir as mybir
from concourse.tile import TileContext
from concourse.bass_utils import run_bass_kernel_spmd

# Problem constants (hardcoded per harness contract).
B = 2
S = 2048
D = 1024
H = 16          # total heads
HD = 64         # head dim
SCALE = D ** (-0.5)
N_CORES = 8
TP = 4          # heads per core
F32 = mybir.dt.float32
F16 = mybir.dt.float16
EXP = mybir.ActivationFunctionType.Exp

KO = D // 128        # 8 contraction tiles over D
ST512 = S // 512     # 4 s-chunks of 512
ST128 = S // 128     # 16 s-tiles of 128
KT = S // 128        # 16 key tiles
QC = 2               # q chunks of 1024
QW = S // QC         # 1024


def _build(phases="ABC"):
    nc = bacc.Bacc("TRN2", target_bir_lowering=False, debug=False,
                   num_devices=N_CORES)

    XDT = F16
    xT = nc.dram_tensor("xT", [D, S], XDT, kind="ExternalInput")
    # wqkv columns: [q 256 | k 256 | v 256], head-major inside each block
    wqkv = nc.dram_tensor("wqkv", [D, 3 * TP * HD], XDT, kind="ExternalInput")
    wp = nc.dram_tensor("wp", [TP * HD, D], F16, kind="ExternalInput")
    bqk = nc.dram_tensor("bqk", [128, 4], F32, kind="ExternalInput")  # q cols *SCALE
    bv = nc.dram_tensor("bv", [1, TP * HD], F16, kind="ExternalInput")
    ones_in = nc.dram_tensor("ones_in", [128, 128], F16, kind="ExternalInput")
    y = nc.dram_tensor("y", [S, D], F32, kind="ExternalOutput")

    with TileContext(nc) as tc, \
         nc.allow_low_precision(reason="fp32r/bf16 rounding for PE operands"):
        with tc.tile_pool(name="pconst", bufs=1) as pc, \
             tc.tile_pool(name="pmain", bufs=1) as pm, \
             tc.tile_pool(name="pp", bufs=1, space="PSUM") as pp:
            # ---- constants / weights (resident) ----
            w_sb = pc.tile([128, KO * 768], XDT, name="w_sb")
            wp_sb = pc.tile([128, 2 * D], F16, name="wp_sb")
            bqk_sb = pc.tile([128, 4], F32, name="bqk_sb")
            bv_sb = pc.tile([1, 256], F16, name="bv_sb")
            ones_sb = pc.tile([128, 128], F16, name="ones_sb")

            # ---- persistent activations ----
            qT_sb = [pm.tile([128, S], F16, name=f"qT{i}") for i in range(2)]
            kT_sb = [pm.tile([128, S], F16, name=f"kT{i}") for i in range(2)]
            v_aug = pm.tile([128, KT * (4 * 65)], F16, name="v_aug")
            attnT = [pm.tile([128, S], F16, name=f"attnT{i}") for i in range(2)]
            v_view = v_aug.rearrange("p (t h c) -> p t h c", h=4, c=65)

            with tc.tile_pool(name="pxT", bufs=1) as px, \
                 tc.tile_pool(name="pwork", bufs=1) as pw:
                xT_sb = [px.tile([128, S], XDT, name=f"xT{i}") for i in range(KO)]
                # interleave x / weight tile loads so (xT[ko], w[ko]) pairs
                # land together and QKV chains progress with arrivals
                for ko in range(KO):
                    nc.sync.dma_start(out=xT_sb[ko][:],
                                      in_=xT[ko * 128:(ko + 1) * 128, :])
                    if ko % 2 == 1:
                        nc.sync.dma_start(
                            out=w_sb.rearrange("p (a c) -> p a c", c=768)
                            [:, ko - 1:ko + 1, :],
                            in_=wqkv[(ko - 1) * 128:(ko + 1) * 128, :]
                            .rearrange("(a p) c -> p a c", p=128))
                # small constants after the bulk stream: their consumers
                # (evac biases, v bias matmul, ones column) all run after
                # the last xT tile lands anyway
                nc.sync.dma_start(out=bqk_sb[:], in_=bqk[:, :])
                nc.sync.dma_start(out=bv_sb[:], in_=bv[:, :])
                nc.sync.dma_start(out=ones_sb[:], in_=ones_in[:, :])
                # softmax-denominator ones column of v_aug (reads ones_sb
                # -> must come after its DMA)
                nc.vector.tensor_copy(
                    v_aug.rearrange("p (t c) -> p t c", c=65)[:, :, 64],
                    ones_sb[:, 0:64])
                for kf in range(2):
                    nc.sync.dma_start(out=wp_sb[:, kf * D:(kf + 1) * D],
                                      in_=wp[kf * 128:(kf + 1) * 128, :])

                def wslice(ko, block, lo, width):
                    off = ko * 768 + block * 256 + lo
                    return w_sb[:, off:off + width]

                # ---------- phase-A building blocks ----------
                def qk_chain(hp, is_q, st):
                    blk = 0 if is_q else 1
                    dst = (qT_sb if is_q else kT_sb)[hp]
                    ps = pp.tile([128, 512], F32, tag="sc", bufs=3,
                                 name="ps_qk")
                    for ko in range(KO):
                        nc.tensor.matmul(
                            ps[:, :],
                            wslice(ko, blk, hp * 128, 128),
                            xT_sb[ko][:, st * 512:(st + 1) * 512],
                            start=(ko == 0), stop=(ko == KO - 1))
                    bias = bqk_sb[:, (0 if is_q else 2) + hp:
                                  (0 if is_q else 2) + hp + 1]
                    if is_q:
                        nc.vector.tensor_scalar(
                            out=dst[:, st * 512:(st + 1) * 512],
                            in0=ps[:, :], scalar1=SCALE, scalar2=bias,
                            op0=mybir.AluOpType.mult, op1=mybir.AluOpType.add)
                    else:
                        nc.vector.tensor_scalar(
                            out=dst[:, st * 512:(st + 1) * 512],
                            in0=ps[:, :], scalar1=bias, scalar2=None,
                            op0=mybir.AluOpType.add)

                def qk_pair(hp):
                    # k strips first: attention consumes kT strip kt ascending
                    for is_q, st in ((False, 0), (True, 0), (True, 1),
                                     (False, 1), (False, 2), (False, 3),
                                     (True, 2), (True, 3)):
                        qk_chain(hp, is_q, st)

                def v_chains(sts):
                    for st in sts:
                        psv = pp.tile([128, 256], F32, tag="sc", bufs=3,
                                      name="ps_v")
                        for ko in range(KO):
                            nc.tensor.matmul(
                                psv[:, :],
                                xT_sb[ko][:, st * 128:(st + 1) * 128],
                                wslice(ko, 2, 0, 256),
                                start=(ko == 0), stop=False)
                        nc.tensor.matmul(psv[:, :], ones_sb[0:1, 0:128],
                                         bv_sb[0:1, :], start=False, stop=True)
                        nc.vector.tensor_copy(
                            v_view[:, st, :, 0:64],
                            psv.rearrange("p (h c) -> p h c", c=64))

                # ---------- phase-B building block ----------
                def attn_unit(h, qc, filler=None, split_tail=False):
                    hp, sub = h // 2, h % 2
                    r0, r1 = sub * 64, sub * 64 + 64

                    def scores(kt):
                        sc = pp.tile([128, QW], F32, tag="sc", bufs=3,
                                     name="sc")
                        for nn in range(2):
                            q0 = qc * QW + nn * 512
                            nc.tensor.matmul(
                                sc[:, nn * 512:(nn + 1) * 512],
                                kT_sb[hp][r0:r1, kt * 128:(kt + 1) * 128],
                                qT_sb[hp][r0:r1, q0:q0 + 512],
                                start=True, stop=True)
                        return sc

                    acc = pp.tile([65, QW], F32, tag="acc", bufs=1,
                                  name="acc")
                    sc_cur = scores(0)
                    for kt in range(KT):
                        sc_next = scores(kt + 1) if kt + 1 < KT else None
                        pt = pw.tile([128, QW], F16, tag="pt", bufs=10,
                                     name="pt")
                        nc.scalar.activation(pt[:, :], sc_cur[:, :], EXP)
                        va = v_aug[:, kt * 260 + h * 65: kt * 260 + h * 65 + 65]
                        for nn in range(2):
                            nc.tensor.matmul(
                                acc[:, nn * 512:(nn + 1) * 512],
                                va, pt[:, nn * 512:(nn + 1) * 512],
                                start=(kt == 0), stop=(kt == KT - 1))
                        if filler is not None:
                            filler(kt)
                        sc_cur = sc_next

                    def tail():
                        outT = pw.tile([65, QW], F16, tag="outT", bufs=3,
                                       name="outT")
                        bc = pp.tile([64, QW], F32, tag="sc", bufs=3,
                                     name="bc")
                        rbc = pw.tile([64, QW], F32, tag="rbc", bufs=2,
                                      name="rbc")
                        if not split_tail:
                            nc.vector.tensor_copy(outT[:, :], acc[:, :])
                            for nn in range(2):
                                nc.tensor.matmul(
                                    bc[:, nn * 512:(nn + 1) * 512],
                                    ones_sb[64:65, 0:64],
                                    outT[64:65, nn * 512:(nn + 1) * 512],
                                    start=True, stop=True)
                            nc.vector.reciprocal(rbc[:, :], bc[:, :])
                            nc.vector.tensor_tensor(
                                out=attnT[hp][r0:r1, qc * QW:(qc + 1) * QW],
                                in0=outT[0:64, :], in1=rbc[:, :],
                                op=mybir.AluOpType.mult)
                            return
                        # final unit: half-width pipelined tail (copies on
                        # the now-idle ScalarE) so the tail projections
                        # unlock earlier
                        for nn in range(2):
                            c0, c1 = nn * 512, (nn + 1) * 512
                            nc.scalar.copy(outT[:, c0:c1], acc[:, c0:c1])
                            nc.tensor.matmul(
                                bc[:, c0:c1], ones_sb[64:65, 0:64],
                                outT[64:65, c0:c1], start=True, stop=True)
                            nc.vector.reciprocal(rbc[:, c0:c1], bc[:, c0:c1])
                            nc.vector.tensor_tensor(
                                out=attnT[hp][r0:r1, qc * QW + c0:qc * QW + c1],
                                in0=outT[0:64, c0:c1], in1=rbc[:, c0:c1],
                                op=mybir.AluOpType.mult)
                    return tail

                # ---------- phase-C building blocks ----------
                def proj_st(py_pool, st, tag="sc"):
                    psy = pp.tile([128, D], F32, tag=tag,
                                  bufs=(1 if tag == "acc" else 3), name="psy")
                    for kf in (1, 0):   # kf=1 first: no early slot grab
                        for nn in range(2):
                            nc.tensor.matmul(
                                psy[:, nn * 512:(nn + 1) * 512],
                                attnT[kf][:, st * 128:(st + 1) * 128],
                                wp_sb[:, kf * D + nn * 512:
                                      kf * D + nn * 512 + 512],
                                start=(kf == 1), stop=(kf == 0))
                    y_sb = py_pool.tile([128, D], F32, tag="y_sb", bufs=5,
                                        name="y_sb")
                    if st < 8:
                        # mid-stream filler: never steal ACT from the exps
                        nc.vector.tensor_copy(y_sb[:, :], psy[:, :])
                    elif st % 2 == 0:
                        nc.vector.tensor_copy(y_sb[:, :], psy[:, :])
                    else:
                        nc.scalar.copy(y_sb[:, :], psy[:, :])
                    nc.sync.dma_start(out=y[st * 128:(st + 1) * 128, :],
                                      in_=y_sb[:, :])

                # ---------- schedule ----------
                from collections import deque
                with tc.tile_pool(name="pyout", bufs=1) as py_pool:
                    has_a = "A" in phases
                    has_b = "B" in phases
                    has_c = "C" in phases

                    bg = deque()

                    def filler(kt):
                        if bg:
                            bg.popleft()()

                    if has_a:
                        # only the chains the first scores/PV need run
                        # before unit 0; everything else is filler work
                        qk_chain(0, False, 0)
                        qk_chain(0, True, 0)
                        qk_chain(0, True, 1)
                        v_chains(list(range(0, 4)))
                    if has_b:
                        # unit 0: k strips 1-3 interleaved with the v strips
                        # EDF order. v_j must pop at iteration <= j-1 (the
                        # filler runs AFTER that iteration's PV in program
                        # order, and program order IS the dependency order);
                        # k strip s must pop before scores(kt=4s) is emitted
                        # at iteration 4s-1.
                        u0 = [("k", 0, False, 1), ("v", 4), ("v", 5),
                              ("v", 6), ("k", 0, False, 2), ("v", 7),
                              ("v", 8), ("v", 9), ("v", 10),
                              ("k", 0, False, 3), ("v", 11), ("v", 12),
                              ("v", 13), ("v", 14), ("v", 15)]
                        for it in u0:
                            if it[0] == "v":
                                bg.append(lambda st=it[1]: v_chains([st]))
                            else:
                                bg.append(lambda a=it[1:]: qk_chain(*a))
                        t = attn_unit(0, 0, filler=filler)
                        # unit 1: prev tail + hp0 qc1 q strips + hp1 chains,
                        # spread with no-op slots to avoid clustering the
                        # sc-slot holds
                        bg.append(t)
                        nop = lambda: None
                        if has_a:
                            for a in ((0, True, 2), (1, False, 0),
                                      (1, False, 1), (1, False, 2),
                                      (1, False, 3), (1, True, 0),
                                      (1, True, 1)):
                                bg.append(lambda a=a: qk_chain(*a))
                                bg.append(nop)
                        t = attn_unit(1, 0, filler=filler)
                        bg.append(t)
                        if has_a:
                            for a in ((0, True, 3), (1, True, 2),
                                      (1, True, 3)):
                                bg.append(lambda a=a: qk_chain(*a))
                                bg.append(nop)
                        t = attn_unit(2, 0, filler=filler)
                        bg.append(t)
                        t = attn_unit(3, 0, filler=filler)
                        bg.append(t)
                        for h in range(TP):
                            # queue the previous block's projections BEFORE
                            # this unit so they pop as its fillers (attnT
                            # qc0 is complete once tail(u3) popped in u4)
                            if has_c and h >= 1:
                                bg.extend(lambda st=st: proj_st(py_pool, st)
                                          for st in range((h - 1) * 3,
                                                          min(h * 3, 8)))
                            t = attn_unit(h, 1, filler=filler,
                                          split_tail=(h == TP - 1))
                            bg.append(t)
                        while bg:
                            bg.popleft()()
                        if has_c:
                            # tail: the acc banks are free -> 4-way parallel
                            for st in range(8, ST128):
                                proj_st(py_pool, st)
                    else:
                        if has_a:
                            v_chains(list(range(4, KT)))
                            qk_pair(1)
                        if has_c:
                            for st in range(ST128):
                                proj_st(py_pool, st)
    nc.compile()
    return nc


_NC_CACHE = None
_last_in_maps = None


def _get_nc():
    global _NC_CACHE
    if _NC_CACHE is None:
        _NC_CACHE = _build()
    return _NC_CACHE


def kernel(x, w_qkv, b_qkv, w_proj, b_proj):
    x = np.ascontiguousarray(np.asarray(x, dtype=np.float32))
    w_qkv = np.asarray(w_qkv, dtype=np.float32)
    b_qkv = np.asarray(b_qkv, dtype=np.float32)
    w_proj = np.asarray(w_proj, dtype=np.float32)
    b_proj = np.asarray(b_proj, dtype=np.float32)

    # Column indices in w_qkv: head h -> q cols [h*192, h*192+64),
    # k cols [h*192+64, h*192+128), v cols [h*192+128, h*192+192).
    ones_np = np.ones((128, 128), np.float16)

    in_maps = []
    for c in range(N_CORES):
        b = c // 4
        g = c % 4
        heads = [4 * g + i for i in range(TP)]
        qcols = np.concatenate([np.arange(h * 192, h * 192 + 64) for h in heads])
        kcols = qcols + 64
        vcols = qcols + 128
        wqkv_c = np.ascontiguousarray(
            np.concatenate([w_qkv[:, qcols], w_qkv[:, kcols], w_qkv[:, vcols]],
                           axis=1))
        wqkv_c = wqkv_c.astype(np.float16)
        bqk_c = np.ascontiguousarray(np.concatenate([
            (b_qkv[qcols] * SCALE).reshape(2, 128).T,
            b_qkv[kcols].reshape(2, 128).T], axis=1))          # [128, 4]
        bv_c = np.ascontiguousarray(b_qkv[vcols].reshape(1, 256)).astype(np.float16)
        # proj rows for this head group: out feature f of head h lives at
        # row h*64+d of w_proj
        prow = np.concatenate([np.arange(h * 64, h * 64 + 64) for h in heads])
        wp_c = np.ascontiguousarray(w_proj[prow, :]).astype(np.float16)
        xT_c = np.ascontiguousarray(x[b].T).astype(np.float16)
        in_maps.append({
            "xT": xT_c, "wqkv": wqkv_c, "wp": wp_c,
            "bqk": bqk_c, "bv": bv_c,
            "ones_in": ones_np,
        })

    global _last_in_maps
    _last_in_maps = in_maps
    nc = _get_nc()
    res = run_bass_kernel_spmd(nc, in_maps, list(range(N_CORES)))
    out = np.zeros((B, S, D), dtype=np.float32)
    for c in range(N_CORES):
        out[c // 4] += res.results[c]["y"]
    out += b_proj
    return out



# revision 5
# speedup vs baseline: 1.2258x; 1.2258x over previous
"""Trainium2 Bass kernel for AltAttention (B=2, S=2048, D=1024, 16 heads).

Distribution over 8 NeuronCores: data-parallel over batch (2) x
tensor-parallel over heads (4 heads/core).

Per-core pipeline (cost-model-aware design):
  - QKV projection chains (fp16 matmuls, PSUM accumulation over D).
  - q/k evacuated to fp8e4m3 in a [64, 2, S] layout (hd split over the two
    DoubleRow k-subtiles; subtile 1 zero-padded) with bias and
    sqrt(1/32)-scale folded in, so the scores matmul runs in fp8 DoubleRow
    mode at 0.5 cycles/row (validated end-to-end error ~9e-3).
  - Scores land as [128 k, 1024 q] PSUM tiles; ScalarE exp stream (the
    ~131 us floor, 128 x [128,1024] activations) converts them to fp16 pt
    tiles in SBUF. The exp stream is the metronome; everything else is
    scheduled into its slack.
  - PV is *flipped*: pt slices [k,q] are the stationary operand, v strips
    [k,64] the moving one, so each accumulation step streams only 64+1
    rows (4x cheaper than streaming q). Accumulators for 8 q-tiles share
    one PSUM bank via memset + start=False accumulation; a parallel
    1-column matmul accumulates the softmax denominators.
  - Normalize (DVE, broadcast multiply by 1/denom), PE-transpose to
    [hd, q], output projection, fp16 partial-y DMA; host sums the 4
    partials per batch and adds b_proj.
"""
import numpy as np

import concourse.bacc as bacc
import concourse.mybir as mybir
from concourse.tile import TileContext
from concourse.bass_utils import run_bass_kernel_spmd

B = 2
S = 2048
D = 1024
H = 16
HD = 64
SCALE = D ** (-0.5)
RS = SCALE ** 0.5          # sqrt-scale folded into both q and k
N_CORES = 8
TP = 4                     # heads per core
F32 = mybir.dt.float32
F16 = mybir.dt.float16
F8 = mybir.dt.float8e4
EXP = mybir.ActivationFunctionType.Exp
DR = mybir.MatmulPerfMode.DoubleRow
MUL = mybir.AluOpType.mult
ADD = mybir.AluOpType.add

KO = D // 128              # 8 contraction tiles over D
KT = S // 128              # 16 key tiles
QC = 2                     # q chunks of 1024
QW = S // QC               # 1024
NQT = QW // 128            # 8 q-subtiles per chunk


def _build():
    nc = bacc.Bacc("TRN2", target_bir_lowering=False, debug=False,
                   num_devices=N_CORES)

    xT = nc.dram_tensor("xT", [D, S], F16, kind="ExternalInput")
    # per-core weight slices, host-prearranged:
    #   wqk: [D, 512] cols = [q h0 h1 h2 h3 | k h0 h1 h2 h3] (64 each)
    #   wv:  [D, 256] cols = v h0..h3
    wqk = nc.dram_tensor("wqk", [D, 512], F16, kind="ExternalInput")
    wv = nc.dram_tensor("wv", [D, 256], F16, kind="ExternalInput")
    wp = nc.dram_tensor("wp", [TP * HD, D], F16, kind="ExternalInput")
    # bqk[:, c]: per-partition bias for chain (kind, hp): cols q-hp0,q-hp1,k-hp0,k-hp1
    bqk = nc.dram_tensor("bqk", [128, 4], F32, kind="ExternalInput")
    bv = nc.dram_tensor("bv", [1, 256], F16, kind="ExternalInput")
    ones_in = nc.dram_tensor("ones_in", [128, 128], F16, kind="ExternalInput")
    eye_in = nc.dram_tensor("eye_in", [128, 128], F16, kind="ExternalInput")
    y = nc.dram_tensor("y", [S, D], F16, kind="ExternalOutput")

    with TileContext(nc) as tc, \
         nc.allow_low_precision(reason="fp16/fp8 PE operands; fp16 partial y"):
        with tc.tile_pool(name="pconst", bufs=1) as pc, \
             tc.tile_pool(name="pmain", bufs=1) as pm, \
             tc.tile_pool(name="pp", bufs=1, space="PSUM") as pp:
            # ---- resident constants / weights ----
            wqk_sb = pc.tile([128, KO, 512], F16, name="wqk_sb")
            wv_sb = pc.tile([128, KO, 256], F16, name="wv_sb")
            wp_sb = pc.tile([128, 2 * D], F16, name="wp_sb")
            bqk_sb = pc.tile([128, 4], F32, name="bqk_sb")
            bv_sb = pc.tile([1, 256], F16, name="bv_sb")
            ones_sb = pc.tile([128, 128], F16, name="ones_sb")
            eye_sb = pc.tile([128, 128], F16, name="eye_sb")
            junk_sb = pc.tile([128, 512], F16, name="junk_sb")
            junk2_sb = pc.tile([128, 128], F16, name="junk2_sb")

            # ---- persistent activations ----
            xT_sb = pm.tile([128, KO, S], F16, name="xT_sb")
            qT = [pm.tile([64, 2, S], F8, name=f"qT{h}") for h in range(TP)]
            kTt = [pm.tile([64, 2, S], F8, name=f"kT{h}") for h in range(TP)]
            v_view = pm.tile([128, KT, TP, HD], F16, name="v_aug")
            attn_sb = [pm.tile([128, NQT, 128], F16, name=f"attn{hp}")
                       for hp in range(2)]
            attnT = [pm.tile([128, S], F16, name=f"attnT{hp}")
                     for hp in range(2)]
            rec_sb = pm.tile([128, 2, NQT], F32, name="rec_sb")

            with tc.tile_pool(name="pwork", bufs=1) as pw:
                # =========== warmup: PE p-state ramp + ACT exp table ========
                nc.gpsimd.memset(junk_sb[:, :], 0.125)
                acc = pp.tile([128, 512], F32, tag="acc", bufs=1, name="acc")
                den = pp.tile([128, NQT, 2], F32, tag="den", bufs=1,
                              name="den")
                for i in range(16):
                    nc.tensor.matmul(acc[:, :], junk_sb[:, 0:128],
                                     junk_sb[:, :], start=True, stop=True)
                nc.scalar.activation(junk2_sb[:, :], junk_sb[:, 0:128], EXP)

                # =========== input DMAs (ordered for earliest first chain) ==
                nc.sync.dma_start(
                    out=wqk_sb[:, :, 0:128],
                    in_=wqk.rearrange("(k p) c -> p k c", p=128)[:, :, 0:128])
                nc.sync.dma_start(
                    out=wqk_sb[:, :, 256:384],
                    in_=wqk.rearrange("(k p) c -> p k c", p=128)[:, :, 256:384])
                nc.sync.dma_start(out=bqk_sb[:], in_=bqk[:, :])
                xTr = xT.rearrange("(k p) s -> p k s", p=128)
                for st in range(4):
                    nc.sync.dma_start(
                        out=xT_sb[:, :, st * 512:(st + 1) * 512],
                        in_=xTr[:, :, st * 512:(st + 1) * 512])
                    if st == 0:
                        nc.sync.dma_start(
                            out=wqk_sb[:, :, 128:256],
                            in_=wqk.rearrange("(k p) c -> p k c", p=128)
                            [:, :, 128:256])
                        nc.sync.dma_start(
                            out=wqk_sb[:, :, 384:512],
                            in_=wqk.rearrange("(k p) c -> p k c", p=128)
                            [:, :, 384:512])
                    if st == 1:
                        nc.sync.dma_start(
                            out=wv_sb[:, :, :],
                            in_=wv.rearrange("(k p) c -> p k c", p=128))
                        nc.sync.dma_start(out=bv_sb[:], in_=bv[:, :])
                        nc.sync.dma_start(out=ones_sb[:], in_=ones_in[:, :])
                        nc.sync.dma_start(out=eye_sb[:], in_=eye_in[:, :])
                for kf in range(2):
                    nc.sync.dma_start(out=wp_sb[:, kf * D:(kf + 1) * D],
                                      in_=wp[kf * 128:(kf + 1) * 128, :])

                # zero the unused DoubleRow k-subtile once (SBUF, Pool ok)
                for h in range(TP):
                    nc.gpsimd.memset(qT[h][0:64, 1, :], 0.0)
                    nc.gpsimd.memset(kTt[h][0:64, 1, :], 0.0)

                # =========== building blocks ================================
                def qk_chain_part(hp, is_q, st, part):
                    """part 0: ko 0-3, part 1: ko 4-7 + evacs. Returns psum
                    tile on part 0 (threaded to part 1 by caller)."""
                    blk = 0 if is_q else 1
                    col = blk * 256 + hp * 128

                    def mk(ps):
                        for ko in (range(4) if part == 0 else range(4, 8)):
                            nc.tensor.matmul(
                                ps[:, :],
                                wqk_sb[:, ko, col:col + 128],
                                xT_sb[:, ko, st * 512:(st + 1) * 512],
                                start=(ko == 0), stop=(ko == KO - 1))
                        if part == 1:
                            dstl = qT if is_q else kTt
                            for sub in range(2):
                                h = 2 * hp + sub
                                bias = bqk_sb[64 * sub:64 * sub + 64,
                                              2 * blk + hp:2 * blk + hp + 1]
                                nc.vector.tensor_scalar(
                                    out=dstl[h][0:64, 0,
                                                st * 512:(st + 1) * 512],
                                    in0=ps[64 * sub:64 * sub + 64, :],
                                    scalar1=RS, scalar2=bias, op0=MUL,
                                    op1=ADD)
                    return mk

                def v_chain_part(st, part):
                    def mk(ps):
                        if part == 0:
                            for ko in range(4):
                                nc.tensor.matmul(
                                    ps[:, 0:256],
                                    xT_sb[:, ko, st * 128:(st + 1) * 128],
                                    wv_sb[:, ko, :],
                                    start=(ko == 0), stop=False)
                        else:
                            for ko in range(4, 8):
                                nc.tensor.matmul(
                                    ps[:, 0:256],
                                    xT_sb[:, ko, st * 128:(st + 1) * 128],
                                    wv_sb[:, ko, :],
                                    start=False, stop=False)
                            nc.tensor.matmul(ps[:, 0:256], ones_sb[0:1, :],
                                             bv_sb[0:1, :], start=False,
                                             stop=True)
                            nc.vector.tensor_copy(
                                v_view[:, st, :, :],
                                ps.rearrange("p (h c) -> p h c", c=HD)
                                [:, 0:4, :])
                    return mk

                def scores(h, qc, kt, sc):
                    for nn in range(2):
                        nc.tensor.matmul(
                            sc[:, nn * 512:(nn + 1) * 512],
                            kTt[h][0:64, :, kt * 128:(kt + 1) * 128],
                            qT[h][0:64, :,
                                  qc * QW + nn * 512:qc * QW + (nn + 1) * 512],
                            start=True, stop=True, perf_mode=DR)

                def pv(h, kt, pt, last):
                    for qt in range(NQT):
                        st_ap = pt[:, qt * 128:(qt + 1) * 128]
                        nc.tensor.matmul(
                            acc[:, qt * HD:(qt + 1) * HD],
                            st_ap, v_view[:, kt, h, :],
                            start=False, stop=last, skip_group_check=True)
                        nc.tensor.matmul(
                            den[:, qt, h % 2:h % 2 + 1],
                            st_ap, ones_sb[:, 0:1],
                            start=False, stop=last, skip_group_check=True)

                def tail_dve(h, qc):
                    hp, sub = h // 2, h % 2
                    nc.vector.reciprocal(rec_sb[:, sub, :], den[:, :, sub])
                    rec_bc = rec_sb[:, sub, :].rearrange(
                        "p (q o) -> p q o", o=1).broadcast_to((128, NQT, HD))
                    nc.vector.tensor_tensor(
                        out=attn_sb[hp][:, :, sub * HD:(sub + 1) * HD],
                        in0=acc.rearrange("p (q c) -> p q c", c=HD),
                        in1=rec_bc, op=MUL)
                    nc.vector.memset(acc[:, :], 0.0)
                    nc.vector.memset(den[:, :, sub], 0.0)

                def transpose_qt(hp, qc, qt):
                    tr = pp.tile([128, 128], F16, tag="ch", bufs=2,
                                 name="tr")
                    nc.tensor.transpose(tr[:, :], attn_sb[hp][:, qt, :],
                                        eye_sb[:, :])
                    nc.vector.tensor_copy(
                        attnT[hp][:, qc * QW + qt * 128:qc * QW + (qt + 1) * 128],
                        tr[:, :])

                def proj_half(st, nn, y_sb, use_act):
                    psy = pp.tile([128, 512], F32, tag="ch", bufs=2,
                                  name="psy")
                    for kf in range(2):
                        nc.tensor.matmul(
                            psy[:, :],
                            attnT[kf][:, st * 128:(st + 1) * 128],
                            wp_sb[:, kf * D + nn * 512:kf * D + nn * 512 + 512],
                            start=(kf == 0), stop=(kf == 1))
                    if use_act:
                        nc.scalar.copy(
                            y_sb[:, nn * 512:(nn + 1) * 512], psy[:, :])
                    else:
                        nc.vector.tensor_copy(
                            y_sb[:, nn * 512:(nn + 1) * 512], psy[:, :])
                    if nn == 1:
                        nc.sync.dma_start(out=y[st * 128:(st + 1) * 128, :],
                                          in_=y_sb[:, :])

                # =========== dependency-gated work queues ===================
                state = {"v_emitted": [False] * KT,
                         "attnT_done": [[False, False], [False, False]],
                         "chains_emitted": set()}

                def chain_tile():
                    return pp.tile([128, 512], F32, tag="ch", bufs=2,
                                   name="chps")

                fillers = []   # list of (cost_ns, guard_fn_or_None, fn)

                def add_chain(hp, is_q, st):
                    p0 = qk_chain_part(hp, is_q, st, 0)
                    p1 = qk_chain_part(hp, is_q, st, 1)
                    box = {}

                    def f0():
                        box['ps'] = chain_tile()
                        p0(box['ps'])

                    def f1():
                        p1(box['ps'])
                        state["chains_emitted"].add((hp, is_q, st))
                    fillers.append([860, None, f0])
                    fillers.append([860, None, f1])

                def add_vchain(st):
                    p0 = v_chain_part(st, 0)
                    p1 = v_chain_part(st, 1)
                    box = {}

                    def f0():
                        box['ps'] = chain_tile()
                        p0(box['ps'])

                    def f1():
                        p1(box['ps'])
                        state["v_emitted"][st] = True
                    fillers.append([430, None, f0])
                    fillers.append([540, None, f1])

                def add_proj(st):
                    qc = st // NQT
                    box = {}

                    def guard():
                        return (state["attnT_done"][0][qc]
                                and state["attnT_done"][1][qc])

                    def f0():
                        box['y'] = pw.tile([128, D], F16, tag="y", bufs=4,
                                           name="y_sb")
                        proj_half(st, 0, box['y'], use_act=(st >= 8))

                    def f1():
                        proj_half(st, 1, box['y'], use_act=False)
                    fillers.append([430, guard, f0])
                    fillers.append([430, guard, f1])

                # =========== schedule =======================================
                # startup chains (gate the first exp):
                #   k-hp0-st0 (kt 0-3), q-hp0-st0, q-hp0-st1  -> unit (h0,qc0)
                for args in ((0, False, 0), (0, True, 0), (0, True, 1)):
                    ps = chain_tile()
                    qk_chain_part(*args, 0)(ps)
                    qk_chain_part(*args, 1)(ps)
                # zero accumulators before first use (real HW PSUM is garbage)
                nc.vector.memset(acc[:, :], 0.0)
                nc.vector.memset(den[:, :, :], 0.0)

                # filler inventory: hard-deadline chains first (they gate the
                # exp stream), then v strips (PV defers until each arrives),
                # then qc1 q strips, then proj.
                add_chain(0, False, 1)
                add_chain(0, False, 2)
                add_chain(0, False, 3)
                add_vchain(0)
                add_vchain(1)
                add_chain(1, False, 0)    # kT h2,h3 st0
                add_vchain(2)
                add_chain(1, True, 0)
                add_vchain(3)
                add_chain(1, True, 1)
                for st in range(4, 8):
                    add_vchain(st)
                add_chain(1, False, 1)
                add_vchain(8)
                add_vchain(9)
                add_chain(1, False, 2)
                add_vchain(10)
                add_vchain(11)
                add_chain(1, False, 3)
                for st in range(12, 16):
                    add_vchain(st)
                add_chain(0, True, 2)
                add_chain(0, True, 3)
                add_chain(1, True, 2)
                add_chain(1, True, 3)
                for st in range(16):
                    add_proj(st)

                # main exp-slot loop
                HEADS = [(qc, h) for qc in range(QC) for h in range(TP)]
                sc_tiles = {}
                pt_tiles = {}

                def emit_scores(uidx, kt):
                    qc, h = HEADS[uidx]
                    sc = pp.tile([128, QW], F32, tag="sc", bufs=2, name="sc")
                    scores(h, qc, kt, sc)
                    sc_tiles[(uidx, kt)] = sc

                def emit_exp(uidx, kt):
                    pt = pw.tile([128, QW], F16, tag="pt", bufs=24, name="pt")
                    nc.scalar.activation(pt[:, :],
                                         sc_tiles.pop((uidx, kt))[:, :], EXP)
                    pt_tiles[(uidx, kt)] = pt

                def emit_pv(uidx, kt):
                    qc, h = HEADS[uidx]
                    pv(h, kt, pt_tiles.pop((uidx, kt)), last=(kt == KT - 1))
                    if kt == KT - 1:
                        tail_dve(h, qc)
                        if h % 2 == 1:
                            for qt in range(NQT):
                                transpose_qt(h // 2, qc, qt)
                            state["attnT_done"][h // 2][qc] = True

                pv_queue = []       # (uidx, kt) FIFO; head gated on v strip
                fi = 0

                def drain(budget, exp_slot):
                    """Emit due PV units and fillers within the PE budget."""
                    nonlocal fi
                    while budget > 0:
                        if pv_queue and state["v_emitted"][pv_queue[0][1]]:
                            pu, pkt = pv_queue.pop(0)
                            emit_pv(pu, pkt)
                            budget -= 220
                            if pkt == KT - 1:
                                budget -= 150
                                if pu % 2 == 1:
                                    budget -= 450
                            continue
                        if fi < len(fillers):
                            cost, guard, fn = fillers[fi]
                            if guard is None or guard():
                                fn()
                                budget -= cost
                                fi += 1
                                continue
                        break

                emit_scores(0, 0)
                n_slots = len(HEADS) * KT
                for slot in range(n_slots):
                    uidx, kt = divmod(slot, KT)
                    if slot + 1 < n_slots:
                        nu, nkt = divmod(slot + 1, KT)
                        emit_scores(nu, nkt)
                    emit_exp(uidx, kt)
                    pv_queue.append((uidx, kt))
                    drain(620, slot)
                # drain everything left
                while pv_queue or fi < len(fillers):
                    before = (len(pv_queue), fi)
                    drain(10 ** 9, n_slots)
                    if (len(pv_queue), fi) == before:
                        raise RuntimeError(
                            f"schedule deadlock: pv={len(pv_queue)} fi={fi}")
    nc.compile()
    return nc


_NC_CACHE = None


def _get_nc():
    global _NC_CACHE
    if _NC_CACHE is None:
        _NC_CACHE = _build()
    return _NC_CACHE


def kernel(x, w_qkv, b_qkv, w_proj, b_proj):
    x = np.ascontiguousarray(np.asarray(x, dtype=np.float32))
    w_qkv = np.asarray(w_qkv, dtype=np.float32)
    b_qkv = np.asarray(b_qkv, dtype=np.float32)
    w_proj = np.asarray(w_proj, dtype=np.float32)
    b_proj = np.asarray(b_proj, dtype=np.float32)

    ones_np = np.ones((128, 128), np.float16)
    eye_np = np.eye(128, dtype=np.float16)

    in_maps = []
    for c in range(N_CORES):
        b = c // 4
        g = c % 4
        heads = [4 * g + i for i in range(TP)]
        # w_qkv cols: head h -> q [h*192, +64), k [+64, +128), v [+128, +192)
        qcols = np.concatenate([np.arange(h * 192, h * 192 + 64)
                                for h in heads])
        kcols = qcols + 64
        vcols = qcols + 128
        wqk_c = np.ascontiguousarray(
            np.concatenate([w_qkv[:, qcols], w_qkv[:, kcols]], axis=1)
        ).astype(np.float16)
        wv_c = np.ascontiguousarray(w_qkv[:, vcols]).astype(np.float16)
        # bias cols: (q,hp0),(q,hp1),(k,hp0),(k,hp1); each [128] = 2 heads x 64
        bq = (b_qkv[qcols] * RS).reshape(2, 128)
        bk = (b_qkv[kcols] * RS).reshape(2, 128)
        bqk_c = np.ascontiguousarray(
            np.stack([bq[0], bq[1], bk[0], bk[1]], axis=1)).astype(np.float32)
        bv_c = np.ascontiguousarray(
            b_qkv[vcols].reshape(1, 256)).astype(np.float16)
        prow = np.concatenate([np.arange(h * 64, h * 64 + 64) for h in heads])
        wp_c = np.ascontiguousarray(w_proj[prow, :]).astype(np.float16)
        xT_c = np.ascontiguousarray(x[b].T).astype(np.float16)
        in_maps.append({
            "xT": xT_c, "wqk": wqk_c, "wv": wv_c, "wp": wp_c,
            "bqk": bqk_c, "bv": bv_c,
            "ones_in": ones_np, "eye_in": eye_np,
        })

    nc = _get_nc()
    res = run_bass_kernel_spmd(nc, in_maps, list(range(N_CORES)))
    out = np.zeros((B, S, D), dtype=np.float32)
    for c in range(N_CORES):
        out[c // 4] += res.results[c]["y"].astype(np.float32)
    out += b_proj
    return out
